# revision 42
# baseline (speedup 1.0000x reference)
"""MixFFN MoE-routing kernel for Trainium2 (8 NeuronCores, token-parallel).

Math (per token block):
    logits = x @ gate_w.T ; probs = softmax(logits); top2 -> ew [N, E] (dense, rows sum to 1)
    CW1 = x @ W1.T ; CW3 = x @ W3.T
    per expert e:
        w1_e = CW1 + (x @ A1e.T) @ B1e.T
        w3_e = CW3 + (x @ A3e.T) @ B3e.T
        h_e  = silu(w1_e) * w3_e
    out = (sum_e ew_e * h_e) @ W2.T + sum_e ((ew_e * h_e) @ A2e.T) @ B2e.T

Active kernel (build_bass_v2, "slot" form): ew has exactly two nonzeros per
token (top-2 of 8 experts), so instead of a dense scan over 8 experts the
kernel computes two "slots" per token.  All 8 experts' rank-16 LoRA factors
are stacked on the E*R = 128 partition axis; a per-slot row-block mask
(M_s[16e+r, n] = [idx_s(n) == e]) zeroes the stacked lora-down projections so
a single K=128 matmul per (slot, proj, dff-tile) yields each token's own
expert delta — 4x fewer LoRA matmuls and ~3.7x fewer elementwise ops than the
dense scan.  Top-2 weights come directly as sigmoid(+-(m1-m2)) of the two
max logits.  The big GEMMs run once: W2 on H = q_0 + q_1, and the B2
correction on masked per-slot U accumulators.  U matmuls are issued one
dff-iteration late so the in-order PE queue never stalls on the elementwise
chain.  (build_bass is the older dense-scan version, kept for reference.)

Sharding: token-parallel.  Each of the 8 cores gets N/8 = 512 tokens and a
replicated copy of all weights; outputs are disjoint row blocks (no
collectives).  All layout transposes / dtype casts are done host-side.

On-chip layout: feature-on-partition ("transposed"), activations [feat, tok].
"""

import numpy as np

# problem dims (hardcoded per harness contract)
N, D, DFF, E, KTOP, R = 4096, 2048, 8192, 8, 2, 16
NCORES = 8
P = 128

_CACHE = {}


def build_bass(D_=D, DFF_=DFF, E_=E, R_=R, NTOK=N // NCORES, repeat=1,
               skip_routing=False, skip_cw=False, skip_delta=False,
               skip_elem=False, skip_u=False, skip_out=False):
    """Build the per-core Bass program (same SPMD program on every core).

    The skip_* flags disable kernel phases for cost-bisection probes;
    all default False (full kernel)."""
    import concourse.bass as bass
    import concourse.mybir as mybir
    from concourse import bacc
    from concourse.tile import TileContext
    from concourse.masks import make_identity

    dt = mybir.dt
    op = mybir.AluOpType
    AF = mybir.ActivationFunctionType

    KD = D_ // P      # contraction tiles over D
    KF = DFF_ // P    # dff tiles
    MD = D_ // P      # output d tiles
    TT = NTOK // P    # token tiles
    ER = E_ * R_      # stacked expert-rank dim (=128 at full size)

    nc = bacc.Bacc("TRN2", target_bir_lowering=False, debug=False)

    # ---- DRAM I/O ----
    # all inputs pre-swizzled host-side so every DMA is 128 contiguous
    # descriptors (partition-major tiles), not thousands of 256B chunks
    x_bf = nc.dram_tensor("x_bf", [P, KD, NTOK], dt.bfloat16, kind="ExternalInput")
    x_f = nc.dram_tensor("x_f", [P, KD, NTOK], dt.float32, kind="ExternalInput")
    gate = nc.dram_tensor("gate", [P, KD, E_], dt.float32, kind="ExternalInput")
    w1t = nc.dram_tensor("w1t", [KF, P, KD, P], dt.bfloat16, kind="ExternalInput")
    w3t = nc.dram_tensor("w3t", [KF, P, KD, P], dt.bfloat16, kind="ExternalInput")
    w2t = nc.dram_tensor("w2t", [MD, P, KF, P], dt.bfloat16, kind="ExternalInput")
    # A1/A3 packed even/odd with 32-aligned expert slots:
    # a1p[:, par, 32*j : 32*j+16] = A1[2*j+par].T  (zeros elsewhere)
    a1p = nc.dram_tensor("a1p", [P, KD, 2, P], dt.bfloat16, kind="ExternalInput")
    a3p = nc.dram_tensor("a3p", [P, KD, 2, P], dt.bfloat16, kind="ExternalInput")
    # B1/B3 packed even/odd with 32-aligned expert slots (rows 32j..32j+16 of
    # plane par hold B[2j+par].T), matching the T-projection psum layout so
    # pairs of delta-matmuls can row-pack via tile_position.
    b1s = nc.dram_tensor("b1s", [KF, P, 2, P], dt.bfloat16, kind="ExternalInput")
    b3s = nc.dram_tensor("b3s", [KF, P, 2, P], dt.bfloat16, kind="ExternalInput")
    a2s = nc.dram_tensor("a2s", [KF, P, ER], dt.bfloat16, kind="ExternalInput")
    b2s = nc.dram_tensor("b2s", [R_, E_, D_], dt.bfloat16, kind="ExternalInput")
    out_t = nc.dram_tensor("out_t", [D_, NTOK], dt.float32, kind="ExternalOutput")

    with TileContext(nc) as tc:
        with (
            tc.tile_pool(name="persist", bufs=1) as persist,
            tc.tile_pool(name="psum_cw", bufs=1, space="PSUM") as ppool_cw,
            tc.tile_pool(name="psum_d", bufs=2, space="PSUM") as ppool_d,
            tc.tile_pool(name="psum_u", bufs=1, space="PSUM") as ppool_u,
            tc.tile_pool(name="dram", bufs=1, space="DRAM") as dpool,
        ):
            for _rep in range(repeat):
                # ---------- persistent tiles (created upfront so the persist
                # pool's footprint is settled before scoped pools stack above) ----
                xbf = persist.tile([P, KD, NTOK], dt.bfloat16)
                nc.sync.dma_start(out=xbf, in_=x_bf[:, :, :])
                ident_f = persist.tile([P, P], dt.float32)
                make_identity(nc, ident_f)
                # H accumulator (bf16) for the whole dff range
                h_big = persist.tile([P, KF, NTOK], dt.bfloat16)
                ewT_sb = persist.tile([E_, NTOK], dt.bfloat16)
                ew_b = []
                for e in range(E_):
                    ewb_t = persist.tile([P, NTOK], dt.bfloat16, tag=f"ewb{e}")
                    ew_b.append(ewb_t)
                t1p, t3p = [None, None], [None, None]
                for par in range(2):
                    t1_t = persist.tile([P, NTOK], dt.bfloat16, tag=f"t1_{par}")
                    t1p[par] = t1_t
                    t3_t = persist.tile([P, NTOK], dt.bfloat16, tag=f"t3_{par}")
                    t3p[par] = t3_t
                uw = []
                for e in range(E_):
                    uw_t = persist.tile([R_, NTOK], dt.bfloat16, tag=f"uw{e}")
                    uw.append(uw_t)

                # ---------- phase 0: routing + lora-down projections ----------
                ew_td = dpool.tile([E_, NTOK], dt.bfloat16)
                p0_cm = tc.tile_pool(name="p0", bufs=3)
                p0 = p0_cm.__enter__()
                xf = p0.tile([P, KD, NTOK], dt.float32, bufs=1)
                nc.sync.dma_start(out=xf, in_=x_f[:, :, :])
                gsb = p0.tile([P, KD, E_], dt.float32, bufs=1)
                nc.sync.dma_start(out=gsb, in_=gate[:, :, :])
                a1sb = p0.tile([P, KD, 2, P], dt.bfloat16, bufs=1)
                nc.sync.dma_start(out=a1sb, in_=a1p[:, :, :, :])
                a3sb = p0.tile([P, KD, 2, P], dt.bfloat16, bufs=1)
                nc.sync.dma_start(out=a3sb, in_=a3p[:, :, :, :])
                ew_pool = p0
                for tt in range(TT) if not skip_routing else ():
                    lg = ppool_d.tile([P, E_], dt.float32, tag="d1")
                    for k in range(KD):
                        nc.tensor.matmul(
                            lg,
                            lhsT=xf[:, k, tt * P:(tt + 1) * P],
                            rhs=gsb[:, k, :],
                            start=(k == 0),
                            stop=(k == KD - 1),
                        )
                    l_sb = ew_pool.tile([P, E_], dt.float32, tag="lsb")
                    nc.vector.tensor_copy(l_sb, lg)
                    m1 = ew_pool.tile([P, 1], dt.float32, tag="m1")
                    nc.vector.reduce_max(m1, l_sb, axis=mybir.AxisListType.X)
                    nm1 = ew_pool.tile([P, 1], dt.float32, tag="nm1")
                    nc.vector.tensor_scalar_mul(nm1, m1, -1.0)
                    mask1 = ew_pool.tile([P, E_], dt.float32, tag="mask1")
                    nc.vector.tensor_scalar(
                        mask1, l_sb, scalar1=m1, scalar2=None, op0=op.is_equal
                    )
                    l2 = ew_pool.tile([P, E_], dt.float32, tag="l2")
                    # l2 = mask1 * (-1e30) + l
                    nc.vector.scalar_tensor_tensor(
                        l2, in0=mask1, scalar=-1e30, in1=l_sb, op0=op.mult, op1=op.add
                    )
                    m2 = ew_pool.tile([P, 1], dt.float32, tag="m2")
                    nc.vector.reduce_max(m2, l2, axis=mybir.AxisListType.X)
                    mask2 = ew_pool.tile([P, E_], dt.float32, tag="mask2")
                    nc.vector.tensor_scalar(
                        mask2, l2, scalar1=m2, scalar2=None, op0=op.is_equal
                    )
                    mask = ew_pool.tile([P, E_], dt.float32, tag="mask")
                    nc.vector.tensor_tensor(mask, mask1, mask2, op=op.add)
                    pexp = ew_pool.tile([P, E_], dt.float32, tag="pexp")
                    nc.scalar.activation(pexp, l_sb, AF.Exp, bias=nm1, scale=1.0)
                    pm = ew_pool.tile([P, E_], dt.float32, tag="pm")
                    nc.vector.tensor_tensor(pm, pexp, mask, op=op.mult)
                    den = ew_pool.tile([P, 1], dt.float32, tag="den")
                    nc.vector.reduce_sum(den, pm, axis=mybir.AxisListType.X)
                    rec = ew_pool.tile([P, 1], dt.float32, tag="rec")
                    nc.vector.reciprocal(rec, den)
                    ewt = ew_pool.tile([P, E_], dt.float32, tag="ewt")
                    nc.vector.tensor_scalar_mul(ewt, pm, rec)
                    # transpose [P, E] -> [E, P] and collect into ewT
                    ewtp = ppool_d.tile([E_, P], dt.float32, tag="d3")
                    nc.tensor.transpose(ewtp, ewt, ident_f)
                    nc.scalar.copy(ewT_sb[:, tt * P:(tt + 1) * P], ewtp)

                if not skip_routing:
                    nc.sync.dma_start(out=ew_td, in_=ewT_sb)
                # broadcast ew rows across partitions: EW_b[e] [P, NTOK]
                for e in range(E_):
                    src = bass.AP(
                        tensor=ew_td.tensor,
                        offset=ew_td.offset + e * NTOK,
                        ap=[[0, P], [1, NTOK]],
                    )
                    nc.sync.dma_start(out=ew_b[e], in_=src)

                # ---------- T1/T3 = stacked per-expert lora-down projections ----------
                # expert 2*j+par sits at rows 32*j..32*j+16 of the `par` chain
                for asb, tlist in ((a1sb, t1p), (a3sb, t3p)) if not skip_delta else ():
                    for par in range(2):
                        tp = ppool_d.tile([P, NTOK], dt.float32, tag="d1")
                        for k in range(KD):
                            nc.tensor.matmul(
                                tp,
                                lhsT=asb[:, k, par, :],
                                rhs=xbf[:, k, :],
                                start=(k == 0),
                                stop=(k == KD - 1),
                            )
                        nc.scalar.copy(tlist[par], tp)

                p0_cm.__exit__(None, None, None)
                stream_cm = tc.tile_pool(name="stream", bufs=2)
                stream = stream_cm.__enter__()

                # ---------- U accumulators (per-expert lora-up of h, unscaled) ----------
                u_ps_a = ppool_u.tile([P, NTOK], dt.float32, tag="ua")
                u_ps_b = ppool_u.tile([P, NTOK], dt.float32, tag="ub")
                u_ps = [u_ps_a, u_ps_b]

                # ---------- main dff loop ----------
                ttc = 0  # round-robin counter for p/q engine assignment
                for kt in range(KF):
                    # CW1/CW3 for this dff tile
                    w1sl = stream.tile([P, KD, P], dt.bfloat16, tag="w1sl")
                    nc.sync.dma_start(out=w1sl, in_=w1t[kt, :, :, :])
                    w3sl = stream.tile([P, KD, P], dt.bfloat16, tag="w3sl")
                    nc.sync.dma_start(out=w3sl, in_=w3t[kt, :, :, :])
                    if not skip_cw:
                        cw1p = ppool_cw.tile([P, NTOK], dt.float32, tag="cw1")
                        cw3p = ppool_cw.tile([P, NTOK], dt.float32, tag="cw3")
                        for k in range(KD):
                            nc.tensor.matmul(
                                cw1p, lhsT=w1sl[:, k, :], rhs=xbf[:, k, :],
                                start=(k == 0), stop=(k == KD - 1),
                            )
                        for k in range(KD):
                            nc.tensor.matmul(
                                cw3p, lhsT=w3sl[:, k, :], rhs=xbf[:, k, :],
                                start=(k == 0), stop=(k == KD - 1),
                            )
                        cw1 = stream.tile([P, NTOK], dt.bfloat16, tag="cw1s")
                        nc.scalar.copy(cw1, cw1p)
                        cw3 = stream.tile([P, NTOK], dt.bfloat16, tag="cw3s")
                        nc.scalar.copy(cw3, cw3p)

                    # per-kt lora weights (even/odd packed, rows 32j hold B[2j+par].T)
                    b1kt = stream.tile([P, 2, P], dt.bfloat16, tag="b1kt")
                    nc.sync.dma_start(out=b1kt, in_=b1s[kt, :, :, :])
                    b3kt = stream.tile([P, 2, P], dt.bfloat16, tag="b3kt")
                    nc.sync.dma_start(out=b3kt, in_=b3s[kt, :, :, :])
                    a2kt = stream.tile([P, ER], dt.bfloat16, tag="a2kt")
                    nc.sync.dma_start(out=a2kt, in_=a2s[kt, :, :])

                    hslice = h_big[:, kt, :]
                    # pairs share a parity and differ in 32-row group, so the two
                    # K=16 delta matmuls of a pair row-pack on the PE
                    for e0, e1 in ((0, 2), (1, 3), (4, 6), (5, 7)) if not (
                            skip_elem and skip_delta) else ():
                        dd = {}
                        for e in (e0, e1) if not skip_delta else ():
                            par, j = e % 2, e // 2
                            r0 = 32 * j
                            d1p = ppool_d.tile([P, NTOK], dt.float32, tag="d1")
                            nc.tensor.matmul(
                                d1p, lhsT=b1kt[r0:r0 + R_, par, :],
                                rhs=t1p[par][r0:r0 + R_, :],
                                start=True, stop=True, tile_position=(r0, 0),
                            )
                            d3p = ppool_d.tile([P, NTOK], dt.float32, tag="d3")
                            nc.tensor.matmul(
                                d3p, lhsT=b3kt[r0:r0 + R_, par, :],
                                rhs=t3p[par][r0:r0 + R_, :],
                                start=True, stop=True, tile_position=(r0, 0),
                            )
                            dd[e] = (d1p, d3p)
                        for e in (e0, e1) if not skip_elem else ():
                            d1p, d3p = dd[e]
                            j = e // 2
                            w1e = stream.tile([P, NTOK], dt.bfloat16, tag="w1e", bufs=3)
                            nc.vector.tensor_tensor(w1e, cw1, d1p, op=op.add)
                            s_e = stream.tile([P, NTOK], dt.bfloat16, tag="s_e", bufs=3)
                            nc.scalar.activation(s_e, w1e, AF.Silu)
                            w3e = stream.tile([P, NTOK], dt.bfloat16, tag="w3e", bufs=3)
                            nc.vector.tensor_tensor(w3e, cw3, d3p, op=op.add)
                            p_e = stream.tile([P, NTOK], dt.bfloat16, tag="p_e", bufs=3)
                            eng = nc.vector if ttc % 4 == 0 else nc.gpsimd
                            ttc += 1
                            eng.tensor_tensor(p_e, s_e, w3e, op=op.mult)
                            # q = ew * p ; first expert writes H directly, rest add
                            if e == 0:
                                eng = nc.gpsimd
                                eng.tensor_tensor(hslice, p_e, ew_b[e], op=op.mult)
                            else:
                                q_e = stream.tile(
                                    [P, NTOK], dt.bfloat16, tag="q_e", bufs=3
                                )
                                eng = nc.vector if ttc % 4 == 0 else nc.gpsimd
                                ttc += 1
                                eng.tensor_tensor(q_e, p_e, ew_b[e], op=op.mult)
                                nc.vector.tensor_tensor(hslice, hslice, q_e, op=op.add)
                            if skip_u:
                                continue
                            # U[e] += A2e.T-contraction of (unscaled) p_e
                            nc.tensor.matmul(
                                u_ps[e % 2][32 * j:32 * j + R_, :],
                                lhsT=a2kt[:, e * R_:(e + 1) * R_],
                                rhs=p_e,
                                start=(kt == 0),
                                stop=(kt == KF - 1),
                                tile_position=(0, 32 * j),
                            )

                # ---------- Uw: apply ew column scaling to U ----------
                for e in range(E_) if not (skip_u or skip_elem) else ():
                    j = e // 2
                    nc.vector.tensor_tensor(
                        uw[e], u_ps[e % 2][32 * j:32 * j + R_, :], ew_b[e][0:R_, :],
                        op=op.mult,
                    )

                # ---------- output GEMM: out = W2 @ H + sum_e B2e @ Uw_e ----------
                KH = KF // 2
                for m in range(MD):
                    outp = ppool_d.tile([P, NTOK], dt.float32, tag="d1")
                    for h in range(2):
                        w2m = stream.tile([P, KH, P], dt.bfloat16, tag="w2m")
                        nc.sync.dma_start(
                            out=w2m, in_=w2t[m, :, h * KH:(h + 1) * KH, :]
                        )
                        if skip_out:
                            continue
                        for kk in range(KH):
                            kt = h * KH + kk
                            nc.tensor.matmul(
                                outp, lhsT=w2m[:, kk, :], rhs=h_big[:, kt, :],
                                start=(kt == 0), stop=False,
                            )
                    b2m = stream.tile([R_, E_, P], dt.bfloat16, tag="b2m")
                    nc.sync.dma_start(out=b2m, in_=b2s[:, :, m * P:(m + 1) * P])
                    if not skip_out:
                        for e in range(E_) if not (skip_u or skip_elem) else ():
                            nc.tensor.matmul(
                                outp, lhsT=b2m[:, e, :], rhs=uw[e],
                                start=False, stop=(e == E_ - 1),
                            )
                        osb = stream.tile([P, NTOK], dt.float32, tag="osb")
                        nc.scalar.copy(osb, outp)
                        nc.sync.dma_start(out=out_t[m * P:(m + 1) * P, :], in_=osb)

                stream_cm.__exit__(None, None, None)

    nc.compile()
    return nc


def build_bass_v2(D_=D, DFF_=DFF, E_=E, R_=R, NTOK=N // NCORES, repeat=1):
    """Slot-restructured kernel: exploit top-2 routing sparsity.

    Instead of scanning all 8 experts densely, compute two "slots" per
    token (its top-1 and top-2 expert).  The per-expert LoRA projections
    become two full-K=128 matmuls against expert-stacked operands with
    token-wise row-block masks selecting each token's expert:

        T1all = A1stack @ x                [E*R=128, NTOK]
        T1m_s = T1all * M_s                (M_s[16e+r, n] = idx_s[n]==e)
        d1_s  = B1stack_kt.T @ T1m_s       (zero rows kill cross terms)
        w1_s  = CW1_kt + d1_s ; h = sum_s w_s * silu(w1_s) * w3_s

    U/B2 use the same mask trick on the ER axis.  Cuts the delta/U/B2
    matmul count 4x and the elementwise op count ~3.7x vs the dense
    expert scan.
    """
    import concourse.bass as bass
    import concourse.mybir as mybir
    from concourse import bacc
    from concourse.tile import TileContext
    from concourse.masks import make_identity

    dt = mybir.dt
    op = mybir.AluOpType
    AF = mybir.ActivationFunctionType

    KD = D_ // P      # contraction tiles over D
    KF = DFF_ // P    # dff tiles
    MD = D_ // P      # output d tiles
    TT = NTOK // P    # token tiles
    ER = E_ * R_      # stacked expert-rank dim (=128 at full size)
    assert ER == P

    nc = bacc.Bacc("TRN2", target_bir_lowering=False, debug=False)

    # ---- DRAM I/O (host-packed; see _prep_inputs_v2) ----
    x_bf = nc.dram_tensor("x_bf", [P, KD, NTOK], dt.bfloat16, kind="ExternalInput")
    x_f = nc.dram_tensor("x_f", [P, KD, NTOK], dt.float32, kind="ExternalInput")
    gate = nc.dram_tensor("gate", [P, KD, E_], dt.float32, kind="ExternalInput")
    w1t = nc.dram_tensor("w1t", [KF, P, KD, P], dt.bfloat16, kind="ExternalInput")
    w3t = nc.dram_tensor("w3t", [KF, P, KD, P], dt.bfloat16, kind="ExternalInput")
    w2t = nc.dram_tensor("w2t", [MD, P, KF, P], dt.bfloat16, kind="ExternalInput")
    # stacked LoRA operands (expert e occupies rows/cols 16e..16e+15)
    a1s = nc.dram_tensor("a1s", [P, KD, ER], dt.bfloat16, kind="ExternalInput")
    a3s = nc.dram_tensor("a3s", [P, KD, ER], dt.bfloat16, kind="ExternalInput")
    b1a = nc.dram_tensor("b1a", [KF, P, P], dt.bfloat16, kind="ExternalInput")
    b3a = nc.dram_tensor("b3a", [KF, P, P], dt.bfloat16, kind="ExternalInput")
    a2a = nc.dram_tensor("a2a", [KF, P, ER], dt.bfloat16, kind="ExternalInput")
    b2a = nc.dram_tensor("b2a", [MD, P, P], dt.bfloat16, kind="ExternalInput")
    # constants: eidx[p] = p // R (expert id of stacked row p);
    # iot8[p, e] = e
    eidx = nc.dram_tensor("eidx", [P, 1], dt.float32, kind="ExternalInput")
    iot8 = nc.dram_tensor("iot8", [P, E_], dt.float32, kind="ExternalInput")
    out_t = nc.dram_tensor("out_t", [D_, NTOK], dt.float32, kind="ExternalOutput")

    with TileContext(nc) as tc:
        with (
            tc.tile_pool(name="persist", bufs=1) as persist,
            tc.tile_pool(name="psum_cw", bufs=1, space="PSUM") as ppool_cw,
            tc.tile_pool(name="psum_d", bufs=2, space="PSUM") as ppool_d,
            tc.tile_pool(name="psum_u", bufs=1, space="PSUM") as ppool_u,
            tc.tile_pool(name="dram", bufs=1, space="DRAM") as dpool,
        ):
            for _rep in range(repeat):
                # ---------- persistent tiles ----------
                xbf = persist.tile([P, KD, NTOK], dt.bfloat16)
                nc.sync.dma_start(out=xbf, in_=x_bf[:, :, :])
                ident_f = persist.tile([P, P], dt.float32)
                make_identity(nc, ident_f)
                h_big = persist.tile([P, KF, NTOK], dt.bfloat16)
                eidx_sb = persist.tile([P, 1], dt.float32, tag="eidx")
                nc.sync.dma_start(out=eidx_sb, in_=eidx[:, :])
                iot8_sb = persist.tile([P, E_], dt.float32, tag="iot8")
                nc.sync.dma_start(out=iot8_sb, in_=iot8[:, :])
                w_b, m_b, t1m, t3m, um = [], [], [], [], []
                for s in range(2):
                    wb_t = persist.tile([P, NTOK], dt.bfloat16, tag=f"wb{s}")
                    w_b.append(wb_t)
                    mb_t = persist.tile([P, NTOK], dt.bfloat16, tag=f"mb{s}")
                    m_b.append(mb_t)
                    t1m_t = persist.tile([P, NTOK], dt.bfloat16, tag=f"t1m{s}")
                    t1m.append(t1m_t)
                    t3m_t = persist.tile([P, NTOK], dt.bfloat16, tag=f"t3m{s}")
                    t3m.append(t3m_t)
                    um_t = persist.tile([P, NTOK], dt.bfloat16, tag=f"um{s}")
                    um.append(um_t)

                # ---------- pre-CW: keep the PE busy while routing's inputs
                # stream in and its (serial) DVE chain runs ----------
                PRE = 3
                pre_cw = []
                for kt in range(PRE):
                    w1sl = persist.tile([P, KD, P], dt.bfloat16, tag=f"w1pre{kt}")
                    nc.sync.dma_start(out=w1sl, in_=w1t[kt, :, :, :])
                    w3sl = persist.tile([P, KD, P], dt.bfloat16, tag=f"w3pre{kt}")
                    nc.sync.dma_start(out=w3sl, in_=w3t[kt, :, :, :])
                    cw1p = ppool_cw.tile([P, NTOK], dt.float32, tag="cw1")
                    cw3p = ppool_cw.tile([P, NTOK], dt.float32, tag="cw3")
                    for k in range(KD):
                        nc.tensor.matmul(
                            cw1p, lhsT=w1sl[:, k, :], rhs=xbf[:, k, :],
                            start=(k == 0), stop=(k == KD - 1),
                        )
                    for k in range(KD):
                        nc.tensor.matmul(
                            cw3p, lhsT=w3sl[:, k, :], rhs=xbf[:, k, :],
                            start=(k == 0), stop=(k == KD - 1),
                        )
                    cw1 = persist.tile([P, NTOK], dt.bfloat16, tag=f"cw1pre{kt}")
                    nc.scalar.copy(cw1, cw1p)
                    cw3 = persist.tile([P, NTOK], dt.bfloat16, tag=f"cw3pre{kt}")
                    nc.scalar.copy(cw3, cw3p)
                    pre_cw.append((cw1, cw3))

                # ---------- phase 0: routing ----------
                ri_d = dpool.tile([2, NTOK], dt.float32)   # idx0, idx1 rows
                rw_d = dpool.tile([2, NTOK], dt.bfloat16)  # w0, w1 rows
                p0_cm = tc.tile_pool(name="p0", bufs=3)
                p0 = p0_cm.__enter__()
                # phase-0 loads ride the Act DMA queue so they don't queue
                # behind the big weight stream on the SP queue
                xf = p0.tile([P, KD, NTOK], dt.float32, bufs=1)
                nc.scalar.dma_start(out=xf, in_=x_f[:, :, :])
                gsb = p0.tile([P, KD, E_], dt.float32, bufs=1)
                nc.scalar.dma_start(out=gsb, in_=gate[:, :, :])
                a1sb = p0.tile([P, KD, ER], dt.bfloat16, bufs=1)
                nc.scalar.dma_start(out=a1sb, in_=a1s[:, :, :])
                a3sb = p0.tile([P, KD, ER], dt.bfloat16, bufs=1)
                nc.scalar.dma_start(out=a3sb, in_=a3s[:, :, :])
                ri_sb = persist.tile([2, NTOK], dt.float32, tag="ri")
                rw_sb = persist.tile([2, NTOK], dt.bfloat16, tag="rw")
                for tt in range(TT):
                    lg = ppool_d.tile([P, E_], dt.float32, tag="d1")
                    for k in range(KD):
                        nc.tensor.matmul(
                            lg,
                            lhsT=xf[:, k, tt * P:(tt + 1) * P],
                            rhs=gsb[:, k, :],
                            start=(k == 0),
                            stop=(k == KD - 1),
                        )
                    l_sb = p0.tile([P, E_], dt.float32, tag="lsb")
                    nc.vector.tensor_copy(l_sb, lg)
                    m1 = p0.tile([P, 1], dt.float32, tag="m1")
                    nc.vector.reduce_max(m1, l_sb, axis=mybir.AxisListType.X)
                    mask1 = p0.tile([P, E_], dt.float32, tag="mask1")
                    nc.vector.tensor_scalar(
                        mask1, l_sb, scalar1=m1, scalar2=None, op0=op.is_equal
                    )
                    l2 = p0.tile([P, E_], dt.float32, tag="l2")
                    nc.vector.scalar_tensor_tensor(
                        l2, in0=mask1, scalar=-1e30, in1=l_sb, op0=op.mult, op1=op.add
                    )
                    m2 = p0.tile([P, 1], dt.float32, tag="m2")
                    nc.vector.reduce_max(m2, l2, axis=mybir.AxisListType.X)
                    mask2 = p0.tile([P, E_], dt.float32, tag="mask2")
                    nc.vector.tensor_scalar(
                        mask2, l2, scalar1=m2, scalar2=None, op0=op.is_equal
                    )
                    # stack_i [P, 2] = (idx0, idx1); stack_w [P, 2] = (w0, w1)
                    stack_i = p0.tile([P, 2], dt.float32, tag="stacki")
                    stack_w = p0.tile([P, 2], dt.float32, tag="stackw")
                    tmp = p0.tile([P, E_], dt.float32, tag="tmp")
                    nc.gpsimd.tensor_tensor(tmp, mask1, iot8_sb, op=op.mult)
                    nc.vector.reduce_sum(
                        stack_i[:, 0:1], tmp, axis=mybir.AxisListType.X)
                    tmp2 = p0.tile([P, E_], dt.float32, tag="tmp2")
                    nc.gpsimd.tensor_tensor(tmp2, mask2, iot8_sb, op=op.mult)
                    nc.vector.reduce_sum(
                        stack_i[:, 1:2], tmp2, axis=mybir.AxisListType.X)
                    dm = p0.tile([P, 1], dt.float32, tag="dm")
                    nc.vector.tensor_tensor(dm, m2, m1, op=op.subtract)
                    # w0 = sigmoid(m1-m2), w1 = sigmoid(m2-m1)
                    nc.scalar.activation(stack_w[:, 0:1], dm, AF.Sigmoid, scale=-1.0)
                    nc.scalar.activation(stack_w[:, 1:2], dm, AF.Sigmoid)
                    stp_i = ppool_d.tile([2, P], dt.float32, tag="d3")
                    nc.tensor.transpose(stp_i, stack_i, ident_f)
                    nc.scalar.copy(ri_sb[:, tt * P:(tt + 1) * P], stp_i)
                    stp_w = ppool_d.tile([2, P], dt.float32, tag="d3")
                    nc.tensor.transpose(stp_w, stack_w, ident_f)
                    nc.scalar.copy(rw_sb[:, tt * P:(tt + 1) * P], stp_w)

                nc.scalar.dma_start(out=ri_d, in_=ri_sb)
                nc.scalar.dma_start(out=rw_d, in_=rw_sb)
                idx_b = []
                for s in range(2):
                    ib = p0.tile([P, NTOK], dt.float32, tag=f"ib{s}", bufs=1)
                    src = bass.AP(
                        tensor=ri_d.tensor,
                        offset=ri_d.offset + s * NTOK,
                        ap=[[0, P], [1, NTOK]],
                    )
                    nc.scalar.dma_start(out=ib, in_=src)
                    idx_b.append(ib)
                    srcw = bass.AP(
                        tensor=rw_d.tensor,
                        offset=rw_d.offset + s * NTOK,
                        ap=[[0, P], [1, NTOK]],
                    )
                    nc.scalar.dma_start(out=w_b[s], in_=srcw)
                    # M_s[p, n] = (idx_s[n] == eidx[p])
                    nc.vector.tensor_scalar(
                        m_b[s], idx_b[s], scalar1=eidx_sb, scalar2=None,
                        op0=op.is_equal,
                    )

                # ---------- stacked lora-down projections + masking ----------
                for asb, tm in ((a1sb, t1m), (a3sb, t3m)):
                    tp = ppool_d.tile([P, NTOK], dt.float32, tag="d1")
                    for k in range(KD):
                        nc.tensor.matmul(
                            tp,
                            lhsT=asb[:, k, :],
                            rhs=xbf[:, k, :],
                            start=(k == 0),
                            stop=(k == KD - 1),
                        )
                    for s in range(2):
                        nc.vector.tensor_tensor(tm[s], tp, m_b[s], op=op.mult)

                p0_cm.__exit__(None, None, None)
                stream_cm = tc.tile_pool(name="stream", bufs=2)
                stream = stream_cm.__enter__()

                # ---------- U accumulators (per-slot, ew-scaled) ----------
                u_ps = []
                for s in range(2):
                    u_ps_t = ppool_u.tile([P, NTOK], dt.float32, tag=f"u{s}")
                    u_ps.append(u_ps_t)

                # ---------- main dff loop ----------
                # U matmuls are issued one iteration late so the in-order PE
                # queue never waits on the elementwise chain producing q.
                prev_q = None
                prev_a2 = None
                for kt in range(KF):
                    if kt >= PRE:
                        w1sl = stream.tile([P, KD, P], dt.bfloat16, tag="w1sl")
                        nc.sync.dma_start(out=w1sl, in_=w1t[kt, :, :, :])
                        w3sl = stream.tile([P, KD, P], dt.bfloat16, tag="w3sl")
                        nc.sync.dma_start(out=w3sl, in_=w3t[kt, :, :, :])
                        cw1p = ppool_cw.tile([P, NTOK], dt.float32, tag="cw1")
                        cw3p = ppool_cw.tile([P, NTOK], dt.float32, tag="cw3")
                        for k in range(KD):
                            nc.tensor.matmul(
                                cw1p, lhsT=w1sl[:, k, :], rhs=xbf[:, k, :],
                                start=(k == 0), stop=(k == KD - 1),
                            )
                        # copy cw1 out as soon as its chain stops: frees the
                        # single-buffered cw1 psum bank before CW1(kt+1) and
                        # unblocks slot-0 elementwise during CW3
                        cw1 = stream.tile([P, NTOK], dt.bfloat16, tag="cw1s")
                        nc.scalar.copy(cw1, cw1p)
                    if prev_q is not None:
                        for s in range(2):
                            nc.tensor.matmul(
                                u_ps[s], lhsT=prev_a2, rhs=prev_q[s],
                                start=(kt == 1), stop=False,
                            )
                    b1kt = stream.tile([P, P], dt.bfloat16, tag="b1kt")
                    nc.scalar.dma_start(out=b1kt, in_=b1a[kt, :, :])
                    b3kt = stream.tile([P, P], dt.bfloat16, tag="b3kt")
                    nc.scalar.dma_start(out=b3kt, in_=b3a[kt, :, :])
                    a2kt = stream.tile([P, ER], dt.bfloat16, tag="a2kt", bufs=3)
                    nc.scalar.dma_start(out=a2kt, in_=a2a[kt, :, :])
                    # deltas issued BEFORE CW3: both slots' w1e+silu overlap
                    # the CW3 chain
                    dd = {}
                    for s in range(2):
                        d1p = ppool_d.tile([P, NTOK], dt.float32, tag="d1")
                        nc.tensor.matmul(
                            d1p, lhsT=b1kt, rhs=t1m[s], start=True, stop=True,
                        )
                        d3p = ppool_d.tile([P, NTOK], dt.float32, tag="d3")
                        nc.tensor.matmul(
                            d3p, lhsT=b3kt, rhs=t3m[s], start=True, stop=True,
                        )
                        dd[s] = (d1p, d3p)
                    if kt >= PRE:
                        for k in range(KD):
                            nc.tensor.matmul(
                                cw3p, lhsT=w3sl[:, k, :], rhs=xbf[:, k, :],
                                start=(k == 0), stop=(k == KD - 1),
                            )
                        cw3 = stream.tile([P, NTOK], dt.bfloat16, tag="cw3s")
                        nc.scalar.copy(cw3, cw3p)
                    else:
                        cw1, cw3 = pre_cw[kt]

                    hslice = h_big[:, kt, :]
                    q = [None, None]
                    for s in range(2):
                        d1p, d3p = dd[s]
                        w1e = stream.tile([P, NTOK], dt.bfloat16, tag="w1e", bufs=3)
                        nc.vector.tensor_tensor(w1e, cw1, d1p, op=op.add)
                        s_e = stream.tile([P, NTOK], dt.bfloat16, tag="s_e", bufs=3)
                        nc.scalar.activation(s_e, w1e, AF.Silu)
                        w3e = stream.tile([P, NTOK], dt.bfloat16, tag="w3e", bufs=3)
                        nc.vector.tensor_tensor(w3e, cw3, d3p, op=op.add)
                        p_e = stream.tile([P, NTOK], dt.bfloat16, tag="p_e", bufs=3)
                        peng = nc.vector if s == 0 else nc.gpsimd
                        peng.tensor_tensor(p_e, s_e, w3e, op=op.mult)
                        q_e = stream.tile([P, NTOK], dt.bfloat16, tag="q_e", bufs=4)
                        qeng = nc.gpsimd if s == 0 else nc.vector
                        qeng.tensor_tensor(q_e, p_e, w_b[s], op=op.mult)
                        q[s] = q_e
                    nc.vector.tensor_tensor(hslice, q[0], q[1], op=op.add)
                    prev_q = q
                    prev_a2 = a2kt
                for s in range(2):
                    nc.tensor.matmul(
                        u_ps[s], lhsT=prev_a2, rhs=prev_q[s],
                        start=False, stop=True,
                    )

                # ---------- mask U ----------
                for s in range(2):
                    nc.vector.tensor_tensor(um[s], u_ps[s], m_b[s], op=op.mult)

                # ---------- output GEMM: out = W2 @ H + sum_s B2stack @ um_s ----------
                KH = KF // 2
                for m in range(MD):
                    outp = ppool_d.tile([P, NTOK], dt.float32, tag="d1")
                    for h in range(2):
                        w2m = stream.tile([P, KH, P], dt.bfloat16, tag="w2m")
                        nc.sync.dma_start(
                            out=w2m, in_=w2t[m, :, h * KH:(h + 1) * KH, :]
                        )
                        for kk in range(KH):
                            kt = h * KH + kk
                            nc.tensor.matmul(
                                outp, lhsT=w2m[:, kk, :], rhs=h_big[:, kt, :],
                                start=(kt == 0), stop=False,
                            )
                    b2m = stream.tile([P, P], dt.bfloat16, tag="b2m")
                    nc.sync.dma_start(out=b2m, in_=b2a[m, :, :])
                    for s in range(2):
                        nc.tensor.matmul(
                            outp, lhsT=b2m, rhs=um[s],
                            start=False, stop=(s == 1),
                        )
                    osb = stream.tile([P, NTOK], dt.float32, tag="osb")
                    nc.scalar.copy(osb, outp)
                    nc.scalar.dma_start(out=out_t[m * P:(m + 1) * P, :], in_=osb)

                stream_cm.__exit__(None, None, None)

    nc.compile()
    return nc


def _prep_inputs_v2(x, W1, W3, W2, gate_w, A1, B1, A3, B3, A2, B2):
    """Host-side packing for build_bass_v2."""
    import ml_dtypes

    bf16 = ml_dtypes.bfloat16
    f32 = np.float32

    xT = np.ascontiguousarray(np.asarray(x, f32).T)            # [D, N]
    A1 = np.asarray(A1, f32)
    A3 = np.asarray(A3, f32)
    B1 = np.asarray(B1, f32)
    B3 = np.asarray(B3, f32)
    A2 = np.asarray(A2, f32)
    B2 = np.asarray(B2, f32)
    E_, R_, D_ = A1.shape
    dff = W1.shape[0]
    ER = E_ * R_

    # a1s [P, KD, ER]: col 16e+r over D = A1[e, r, :]
    a1sk = _sw_d(np.ascontiguousarray(A1.transpose(2, 0, 1).reshape(D_, ER)))
    a3sk = _sw_d(np.ascontiguousarray(A3.transpose(2, 0, 1).reshape(D_, ER)))
    # b1a [KF, ER, 128]: b1a[kt, 16e+r, f] = B1[e, kt*128+f, r]
    b1ak = np.ascontiguousarray(
        B1.transpose(0, 2, 1).reshape(ER, dff // P, P).transpose(1, 0, 2))
    b3ak = np.ascontiguousarray(
        B3.transpose(0, 2, 1).reshape(ER, dff // P, P).transpose(1, 0, 2))
    # a2a [KF, 128, ER]: a2a[kt, f, 16e+r] = A2[e, r, kt*128+f]
    a2ak = np.ascontiguousarray(
        A2.reshape(ER, dff).T.reshape(dff // P, P, ER))
    # b2a [MD, ER, 128]: b2a[m, 16e+r, d] = B2[e, m*128+d, r]
    b2ak = np.ascontiguousarray(
        B2.transpose(0, 2, 1).reshape(ER, D_).reshape(ER, D_ // P, P)
        .transpose(1, 0, 2))
    eidx = (np.arange(P, dtype=np.int64) // R_).astype(f32).reshape(P, 1)
    iot8 = np.broadcast_to(
        np.arange(E_, dtype=f32), (P, E_)).copy()

    shared = {
        "gate": _sw_d(np.ascontiguousarray(np.asarray(gate_w, f32).T)),
        "w1t": _pack_w_ktiles(np.asarray(W1, f32).T.astype(bf16)),
        "w3t": _pack_w_ktiles(np.asarray(W3, f32).T.astype(bf16)),
        "w2t": _pack_w_ktiles(np.asarray(W2, f32).T.astype(bf16)),
        "a1s": a1sk.astype(bf16),
        "a3s": a3sk.astype(bf16),
        "b1a": b1ak.astype(bf16),
        "b3a": b3ak.astype(bf16),
        "a2a": a2ak.astype(bf16),
        "b2a": b2ak.astype(bf16),
        "eidx": eidx,
        "iot8": iot8,
    }
    ntok = xT.shape[1] // NCORES
    in_maps = []
    for c in range(NCORES):
        sl = np.ascontiguousarray(xT[:, c * ntok:(c + 1) * ntok])
        m = dict(shared)
        m["x_f"] = _sw_d(sl)
        m["x_bf"] = _sw_d(sl.astype(bf16))
        in_maps.append(m)
    return in_maps


def _sw_d(arr):
    """[D, ...] -> [P, KD, ...] partition-major swizzle (d = k*128 + p)."""
    D_ = arr.shape[0]
    rest = arr.shape[1:]
    return np.ascontiguousarray(
        arr.reshape(D_ // 128, 128, *rest).swapaxes(0, 1)
    )


def _pack_a_evenodd(A):
    """A [E, R, D] -> [P, KD, 2, 128] with A[2j+par].T at [:, :, par, 32j:+16]."""
    E_, R_, D_ = A.shape
    out = np.zeros((D_, 2, 128), A.dtype)
    for e in range(E_):
        par, j = e % 2, e // 2
        out[:, par, 32 * j:32 * j + R_] = A[e].T
    return _sw_d(out)


def _pack_b_evenodd(B):
    """B [E, F, R] -> [KF, 128, 2, 128]: B[2j+par].T kt-tiles at
    [kt, 32j:32j+16, par, :]."""
    E_, F_, R_ = B.shape
    out = np.zeros((128, 2, F_), B.dtype)
    for e in range(E_):
        par, j = e % 2, e // 2
        out[32 * j:32 * j + R_, par, :] = B[e].T
    # [row, par, (kt n)] -> [kt, row, par, n]
    return np.ascontiguousarray(
        out.reshape(128, 2, F_ // 128, 128).transpose(2, 0, 1, 3)
    )


def _pack_w_ktiles(WT):
    """WT [K, M] (contraction-major) -> [MT, P, KT, P] where
    out[mt, p, kt, n] = WT[kt*128+p, mt*128+n] — per-(mt) slab is
    partition-major with [KT, 128] contiguous per partition."""
    K_, M_ = WT.shape
    return np.ascontiguousarray(
        WT.reshape(K_ // 128, 128, M_ // 128, 128).transpose(2, 1, 0, 3)
    )


def _prep_inputs(x, W1, W3, W2, gate_w, A1, B1, A3, B3, A2, B2):
    """Host-side packing: transposes + casts, shared across cores."""
    import ml_dtypes

    bf16 = ml_dtypes.bfloat16
    f32 = np.float32

    xT = np.ascontiguousarray(np.asarray(x, f32).T)            # [D, N]
    dff = W1.shape[0]
    shared = {
        "gate": _sw_d(np.ascontiguousarray(np.asarray(gate_w, f32).T)),
        "w1t": _pack_w_ktiles(np.asarray(W1, f32).T.astype(bf16)),
        "w3t": _pack_w_ktiles(np.asarray(W3, f32).T.astype(bf16)),
        "w2t": _pack_w_ktiles(np.asarray(W2, f32).T.astype(bf16)),
        "a1p": _pack_a_evenodd(np.asarray(A1, f32)).astype(bf16),
        "a3p": _pack_a_evenodd(np.asarray(A3, f32)).astype(bf16),
        "b1s": _pack_b_evenodd(np.asarray(B1, f32)).astype(bf16),
        "b3s": _pack_b_evenodd(np.asarray(B3, f32)).astype(bf16),
        "a2s": np.ascontiguousarray(
            np.asarray(A2, f32).transpose(2, 0, 1).reshape(dff // 128, 128, -1)
        ).astype(bf16),
        "b2s": np.ascontiguousarray(np.asarray(B2, f32).transpose(2, 0, 1)).astype(bf16),
    }
    ntok = xT.shape[1] // NCORES
    in_maps = []
    for c in range(NCORES):
        sl = np.ascontiguousarray(xT[:, c * ntok:(c + 1) * ntok])
        m = dict(shared)
        m["x_f"] = _sw_d(sl)
        m["x_bf"] = _sw_d(sl.astype(bf16))
        in_maps.append(m)
    return in_maps


USE_V2 = True


def _build():
    return build_bass_v2() if USE_V2 else build_bass()


def _prep(inputs):
    fn = _prep_inputs_v2 if USE_V2 else _prep_inputs
    return fn(**inputs)


def _ensure_compiled():
    if "exec" not in _CACHE:
        _CACHE["exec"] = _make_exec(_build())
    return _CACHE["exec"]


def _make_exec(nc):
    """Build a jitted 8-core shard_map executor for a Bass program.

    Mirrors concourse.bass2jax.run_bass_via_pjrt, but caches the jitted
    callable and keeps real inputs un-donated so device buffers can be
    reused across calls (for timing)."""
    import jax
    import concourse.mybir as mybir
    from concourse import bass2jax
    from jax.experimental.shard_map import shard_map
    from jax.sharding import Mesh, PartitionSpec

    bass2jax.install_neuronx_cc_hook()

    partition_name = (
        nc.partition_id_tensor.name if nc.partition_id_tensor else None
    )
    in_names, out_names, out_avals, zero_outs = [], [], [], []
    for alloc in nc.m.functions[0].allocations:
        if not isinstance(alloc, mybir.MemoryLocationSet):
            continue
        name = alloc.memorylocations[0].name
        if alloc.kind == "ExternalInput":
            if name != partition_name:
                in_names.append(name)
        elif alloc.kind == "ExternalOutput":
            np_dtype = mybir.dt.np(alloc.dtype)
            out_names.append(name)
            out_avals.append(
                jax.core.ShapedArray(tuple(alloc.tensor_shape), np_dtype)
            )
            zero_outs.append(np.zeros(tuple(alloc.tensor_shape), np_dtype))

    n_params = len(in_names)
    n_outs = len(out_names)
    all_names = in_names + out_names
    if partition_name is not None:
        all_names = all_names + [partition_name]

    def _body(*args):
        operands = list(args)
        if partition_name is not None:
            operands.append(bass2jax.partition_id_tensor())
        outs = bass2jax._bass_exec_p.bind(
            *operands,
            out_avals=tuple(out_avals),
            in_names=tuple(all_names),
            out_names=tuple(out_names),
            lowering_input_output_aliases=(),
            sim_require_finite=True,
            sim_require_nnan=True,
            nc=nc,
        )
        return tuple(outs)

    devices = jax.devices()[:NCORES]
    mesh = Mesh(np.asarray(devices), ("core",))
    in_specs = (PartitionSpec("core"),) * (n_params + n_outs)
    out_specs = (PartitionSpec("core"),) * n_outs
    donate = tuple(range(n_params, n_params + n_outs))
    sharded = jax.jit(
        shard_map(
            _body, mesh=mesh, in_specs=in_specs, out_specs=out_specs,
            check_rep=False,
        ),
        donate_argnums=donate,
        keep_unused=True,
    )
    ctx = {
        "fn": sharded,
        "body": _body,
        "n_operands": n_params + n_outs,
        "in_names": in_names,
        "out_names": out_names,
        "zero_outs": zero_outs,
        "mesh": mesh,
    }
    return ctx


def _concat_inputs(in_maps, in_names):
    return [
        np.concatenate([in_maps[c][nm] for c in range(NCORES)], axis=0)
        for nm in in_names
    ]


def _run(ctx, concat_in):
    zeros = [
        np.zeros((NCORES * z.shape[0], *z.shape[1:]), z.dtype)
        for z in ctx["zero_outs"]
    ]
    return ctx["fn"](*concat_in, *zeros)


def kernel(x, W1, W3, W2, gate_w, A1, B1, A3, B3, A2, B2):
    ctx = _ensure_compiled()
    in_maps = _prep(dict(x=x, W1=W1, W3=W3, W2=W2, gate_w=gate_w, A1=A1,
                         B1=B1, A3=A3, B3=B3, A2=A2, B2=B2))
    concat_in = _concat_inputs(in_maps, ctx["in_names"])
    out_arrs = _run(ctx, concat_in)
    ntok = N // NCORES
    res = np.asarray(out_arrs[ctx["out_names"].index("out_t")])
    res = res.reshape(NCORES, D, ntok)
    out = np.empty((N, D), np.float32)
    for c in range(NCORES):
        out[c * ntok:(c + 1) * ntok, :] = res[c].T
    return out


def time_device(inputs, iters=5, ctx=None, batch=64, inner_rep=8):
    """Steady-state per-execution HW time of the kernel.

    Methodology: the kernel body is compiled with `inner_rep`
    back-to-back repetitions inside one NEFF (each repetition is the
    complete kernel: all DMAs from DRAM, routing, GEMMs, output
    store), and `batch` such executables are launched back-to-back
    with a single sync at the end.  Per-execution time is
    total / (batch * inner_rep).  This amortizes the client<->device
    tunnel round-trip (~70 ms here, measured identical for an empty
    kernel) and the per-dispatch processing tax — both dispatch
    latency, not HW execution time.  Returns the min over `iters`
    measurements.
    """
    import time as _time

    import jax
    from jax.experimental.shard_map import shard_map
    from jax.sharding import NamedSharding, PartitionSpec, Mesh

    key = f"exec_rep{inner_rep}"
    if key not in _CACHE:
        build = build_bass_v2 if USE_V2 else build_bass
        _CACHE[key] = _make_exec(build(repeat=inner_rep))
    ctx = _CACHE[key]
    if "fn_nodonate" not in ctx:
        ctx["fn_nodonate"] = jax.jit(
            shard_map(
                ctx["body"], mesh=ctx["mesh"],
                in_specs=(PartitionSpec("core"),) * ctx["n_operands"],
                out_specs=(PartitionSpec("core"),) * len(ctx["out_names"]),
                check_rep=False,
            ),
            keep_unused=True,
        )
    fn = ctx["fn_nodonate"]
    in_maps = _prep(inputs)
    concat_in = _concat_inputs(in_maps, ctx["in_names"])
    zeros = [
        np.zeros((NCORES * z.shape[0], *z.shape[1:]), z.dtype)
        for z in ctx["zero_outs"]
    ]
    sh = NamedSharding(ctx["mesh"], PartitionSpec("core"))
    dev = [jax.device_put(a, sh) for a in (concat_in + zeros)]
    jax.block_until_ready(fn(*dev))  # warmup/compile
    times = []
    for _ in range(iters):
        t0 = _time.perf_counter()
        out = None
        for _ in range(batch):
            out = fn(*dev)
        # device queues are FIFO: the last launch finishing implies all
        # earlier launches finished
        jax.block_until_ready(out)
        times.append((_time.perf_counter() - t0) / (batch * inner_rep))
    return min(times)

